# revision 10
# baseline (speedup 1.0000x reference)
"""MoE NaiveGate kernel for Trainium2 (8 NeuronCores, data-parallel over tokens).

Computes, for inp [16384, 2048] f32, W [64, 2048] f32, b [64] f32:
    gate = inp @ W.T + b            # [16384, 64]
    top_val, top_idx = top_k(gate, 2)
    gate_score = softmax(top_val)[:, None, :]   # [16384, 1, 2]
    returns (top_idx.reshape(-1) int32, gate_score f32)

Sharding: token dim split 8 ways (2048 tokens/core); W, b replicated.

Per-core pipeline (all within one NeuronCore, Tile-scheduled):
  - DMA 4MB natural tiles [128 tok, 4, 2048 d] (fully contiguous rows).
  - PE transposes 128x128 blocks (inp must be d-major for the matmul
    contraction; fp32 DMA transpose does not exist on trn2).
  - f32r matmuls with W.T stationary: gate.T [64, 512] accumulated in PSUM.
  - PE transpose of gate.T back to [tok, 64], bias added during PSUM drain.
  - HW MAX8/MAX_INDEX top-k, batched 2-way softmax, two small output DMAs.
"""

import sys

for _p in ("/opt/trn_rl_repo",):
    if _p not in sys.path:
        sys.path.insert(0, _p)

import numpy as np

import concourse.bass as bass
import concourse.bacc as bacc
import concourse.mybir as mybir
import concourse.tile as tile
from concourse.masks import make_identity

N_CORES = 8
TOKENS = 16384
D = 2048
E = 64
TOP_K = 2

T_CORE = TOKENS // N_CORES  # 2048 tokens per core
GROUP = 512                 # tokens per matmul moving operand
N_GROUPS = T_CORE // GROUP  # 4
SUBT = GROUP // 128         # 4 token-subtiles per group
KCH = D // 128              # 16 contraction chunks
N_TILES = T_CORE // 128     # 16 token tiles per core

f32 = mybir.dt.float32
f32r = mybir.dt.float32r
u32 = mybir.dt.uint32


def build_gate_kernel(matmul_mode: str = "f32r") -> bass.Bass:
    """matmul_mode:
    'f32r'    - fp32 data through the fast PE path (1 cyc/row at N=512)
    'f32'     - exact fp32 (4 cyc/row)
    'f32pack' - exact fp32, two col-packed concurrent matmuls (~2 cyc/row),
                even/odd k-chunks accumulate into partitions 0:64 / 64:128
                of one PSUM bank and are summed during the drain.
    """
    nc = bacc.Bacc(None)

    inp = nc.declare_dram_parameter("inp", [T_CORE, D], f32, isOutput=False)
    W = nc.declare_dram_parameter("W", [E, D], f32, isOutput=False)
    b = nc.declare_dram_parameter("b", [E], f32, isOutput=False)
    out_idx = nc.declare_dram_parameter("out_idx", [T_CORE, TOP_K], u32, isOutput=True)
    out_score = nc.declare_dram_parameter("out_score", [T_CORE, TOP_K], f32, isOutput=True)

    def mm_cast(ap):
        return ap.bitcast(f32r) if matmul_mode == "f32r" else ap

    with tile.TileContext(nc) as tc:
        with (
            tc.tile_pool(name="const", bufs=1) as const_pool,
            tc.tile_pool(name="nat", bufs=2) as nat_pool,
            tc.tile_pool(name="inpT", bufs=2) as inpT_pool,
            tc.tile_pool(name="small", bufs=4) as small_pool,
            tc.tile_pool(name="acc", bufs=1) as acc_pool,
            tc.tile_pool(name="ps_t", bufs=3, space="PSUM") as ps_t_pool,
            tc.tile_pool(name="ps_g", bufs=2, space="PSUM") as ps_g_pool,
            tc.tile_pool(name="ps_s", bufs=2, space="PSUM") as ps_s_pool,
        ):
            # ---- constants / prologue ----
            ident = const_pool.tile([128, 128], f32)
            make_identity(nc, ident)

            # bias replicated to all 128 partitions during DMA
            bias_sb = const_pool.tile([128, E], f32)
            b_ap = b[:]
            bias_bcast = bass.AP(
                tensor=b_ap.tensor, offset=b_ap.offset, ap=[[0, 128]] + list(b_ap.ap)
            )
            nc.gpsimd.dma_start(out=bias_sb[:], in_=bias_bcast)

            # W [64, 2048] -> WT_sb [128, kch, 64] (d-major)
            # For f32r mode the drain copies write through an f32r-bitcast AP:
            # walrus requires f32r matmul inputs to be rounded by the producer.
            w_sb = const_pool.tile([E, D], f32)
            nc.sync.dma_start(out=w_sb[:], in_=W[:, :])
            WT_sb = const_pool.tile([128, KCH, E], f32)
            for k in range(KCH):
                pw = ps_s_pool.tile([128, E], f32, tag="ps_small")
                nc.tensor.transpose(pw[:], w_sb[:, k * 128:(k + 1) * 128], ident[:E, :E])
                nc.vector.tensor_copy(mm_cast(WT_sb[:, k, :]), pw[:])

            # per-core collected top-8 values / indices
            vals8 = acc_pool.tile([128, N_TILES, 8], f32)
            idx8 = acc_pool.tile([128, N_TILES, 8], u32)

            pending = None  # deferred gate epilogue of the previous group

            def gate_epilogue(g, gate_ps):
                # gate.T [64, 512] psum -> sbuf, then 4 PE transposes back to
                # [128 tok, 64] and per-tile top-k.
                gsbT = small_pool.tile([E, GROUP], f32, tag="gsbT")
                if matmul_mode == "f32pack":
                    nc.vector.tensor_add(gsbT[:], gate_ps[:E, :], gate_ps[E:2 * E, :])
                else:
                    nc.vector.tensor_copy(gsbT[:], gate_ps[:E, :])
                for s in range(SUBT):
                    ti = g * SUBT + s
                    pt = ps_s_pool.tile([128, E], f32, tag="ps_small")
                    nc.tensor.transpose(
                        pt[:], gsbT[:, s * 128:(s + 1) * 128], ident[:E, :E]
                    )
                    gate_sb = small_pool.tile([128, E], f32, tag="gate_sb")
                    nc.vector.tensor_add(gate_sb[:], pt[:], bias_sb[:])
                    nc.vector.max(out=vals8[:, ti], in_=gate_sb[:])
                    nc.vector.max_index(
                        out=idx8[:, ti], in_max=vals8[:, ti], in_values=gate_sb[:]
                    )

            # ---- main loop over 512-token groups ----
            for g in range(N_GROUPS):
                nat = nat_pool.tile([128, SUBT, D], f32)
                nc.sync.dma_start(
                    out=nat[:],
                    in_=inp[g * GROUP:(g + 1) * GROUP, :].rearrange(
                        "(s p) d -> p s d", p=128
                    ),
                )

                inpT = inpT_pool.tile([128, KCH, GROUP], f32)
                for k in range(KCH):
                    pt = ps_t_pool.tile([128, GROUP], f32)
                    for s in range(SUBT):
                        nc.tensor.matmul(
                            pt[:, s * 128:(s + 1) * 128],
                            nat[:, s, k * 128:(k + 1) * 128],
                            ident[:],
                            is_transpose=True,
                            start=(s == 0),
                            stop=(s == SUBT - 1),
                        )
                    # drain psum -> sbuf; in non-f32r modes give every 4th
                    # copy to ACT to offload the vector engine a bit
                    if matmul_mode != "f32r" and k % 4 == 3:
                        nc.scalar.copy(inpT[:, k, :], pt[:])
                    else:
                        nc.vector.tensor_copy(mm_cast(inpT[:, k, :]), pt[:])

                if pending is not None:
                    gate_epilogue(*pending)
                    pending = None

                if matmul_mode == "f32pack":
                    # two concurrent col-packed fp32 matmuls: even k-chunks
                    # accumulate into psum partitions 0:64, odd into 64:128
                    gate_ps = ps_g_pool.tile([2 * E, GROUP], f32, tag="gate_ps")
                    for k in range(KCH):
                        half = k % 2
                        nc.tensor.matmul(
                            gate_ps[half * E:(half + 1) * E, :],
                            WT_sb[:, k, :],
                            inpT[:, k, :],
                            start=(k < 2),
                            stop=(k >= KCH - 2),
                            skip_group_check=True,
                        )
                else:
                    gate_ps = ps_g_pool.tile([E, GROUP], f32, tag="gate_ps")
                    for k in range(KCH):
                        nc.tensor.matmul(
                            gate_ps[:],
                            mm_cast(WT_sb[:, k, :]),
                            mm_cast(inpT[:, k, :]),
                            start=(k == 0),
                            stop=(k == KCH - 1),
                        )
                pending = (g, gate_ps)

            gate_epilogue(*pending)

            # ---- batched softmax over the two top logits ----
            v0 = vals8[:, :, 0]
            v1 = vals8[:, :, 1]
            d_t = small_pool.tile([128, N_TILES], f32, tag="soft")
            e_t = small_pool.tile([128, N_TILES], f32, tag="soft")
            den_t = small_pool.tile([128, N_TILES], f32, tag="soft")
            score_sb = acc_pool.tile([128, N_TILES, TOP_K], f32)
            nc.vector.tensor_sub(d_t[:], v1, v0)
            nc.scalar.activation(e_t[:], d_t[:], mybir.ActivationFunctionType.Exp)
            nc.vector.tensor_scalar_add(den_t[:], e_t[:], 1.0)
            nc.vector.reciprocal(score_sb[:, :, 0], den_t[:])
            nc.vector.tensor_mul(score_sb[:, :, 1], e_t[:], score_sb[:, :, 0])

            # ---- outputs ----
            nc.sync.dma_start(
                out=out_idx.rearrange("(tl p) k -> p tl k", p=128),
                in_=idx8[:, :, 0:TOP_K],
            )
            nc.sync.dma_start(
                out=out_score.rearrange("(tl p) k -> p tl k", p=128),
                in_=score_sb[:],
            )

    nc.finalize()
    return nc


_NC_CACHE: dict = {}


def _get_nc(matmul_mode: str) -> bass.Bass:
    if matmul_mode not in _NC_CACHE:
        _NC_CACHE[matmul_mode] = build_gate_kernel(matmul_mode)
    return _NC_CACHE[matmul_mode]


def kernel(inp, W, b, matmul_mode: str = "f32r", trace: bool = False, **run_kwargs):
    from concourse.bass_utils import run_bass_kernel_spmd

    inp = np.ascontiguousarray(np.asarray(inp, dtype=np.float32))
    W = np.ascontiguousarray(np.asarray(W, dtype=np.float32))
    b = np.ascontiguousarray(np.asarray(b, dtype=np.float32))
    assert inp.shape == (TOKENS, D) and W.shape == (E, D) and b.shape == (E,)

    nc = _get_nc(matmul_mode)
    in_maps = [
        {"inp": inp[c * T_CORE:(c + 1) * T_CORE], "W": W, "b": b}
        for c in range(N_CORES)
    ]
    res = run_bass_kernel_spmd(
        nc, in_maps, core_ids=list(range(N_CORES)), trace=trace, **run_kwargs
    )
    kernel.last_result = res

    idx = np.concatenate([res.results[c]["out_idx"] for c in range(N_CORES)], axis=0)
    score = np.concatenate(
        [res.results[c]["out_score"] for c in range(N_CORES)], axis=0
    )
    gate_top_k_idx = idx.astype(np.int32).reshape(-1)
    gate_score = score.reshape(TOKENS, 1, TOP_K).astype(np.float32)
    return (gate_top_k_idx, gate_score)
